# revision 29
# baseline (speedup 1.0000x reference)
"""GCGRUCell Trainium2 kernel — 8-core SPMD, fp8 aggregation path.

Math (per reference):
  value = sigmoid(cat([x, h]) @ W_fc + b_fc);  r, u = split(value)
  X0 = cat([x, r*h])                                (B, N, D)   D=66
  Y  = X0 @ Wg_odd                                  (B, N, U)
  X1 = S @ Y          (segment_sum over E edges, commuted with Wg_odd)
  c  = tanh(X0 @ Wg_even + X1 + b_g)
  out = u * hx + (1-u) * c
Sharding: nodes 1250/core (10 dest blocks of 128, greedily balanced);
all B=16 batches per core for the aggregation (gathered rows are
B*U=1024B fp8).

The gather is per-DESCRIPTOR-latency bound (~4.8 ns/desc aggregate +
~3.3 ns/KB measured), so phase B dedups sources per gather GROUP of 3
dest blocks: each group's ~6k edges reference ~4.6k distinct sources;
one descriptor per DISTINCT source, and the per-(chunk, dest-block)
one-hot fp8 DoubleRow matrices absorb the duplicate fan-out (a slot may
carry several dests / accumulated parallel edges).  3 PSUM accumulators
(one per block in the group) live through the group's chunks; Z0 joins
the same accumulation from the persisted catT (16 bf16 matmuls/block).

Phase A per block/half: r logits + sigmoid on ACT, u logits via
transposing matmuls copied as LOGITS (bf16, Pool engine) into usb —
u's sigmoid runs in phase C on the idle ACT engine; r*h in place into
catT; y matmuls with a DVE PSUM->fp8 copy.  Y rows are all-gathered (5
pieces, overlapping A) into a (10240, 1024) fp8 y_tab.  oh/idx/hx are
streamed per group in phase B (double-buffered) instead of prefetched.

Row order: tile t in [0,16) = batch, 128 nodes per tile; feature order
[h (0:64), xi (64:66), ones (66)]; biases ride the ones row.
"""

from contextlib import nullcontext

import numpy as np
import ml_dtypes
import concourse.bass as bass
import concourse.bacc as bacc
import concourse.mybir as mybir
import concourse.tile as tile
from bass_rust import add_dep_helper
from concourse.bass_utils import run_bass_kernel_spmd

F32 = mybir.dt.float32
BF16 = mybir.dt.bfloat16
FP8 = mybir.dt.float8e4
I16 = mybir.dt.int16

NCORES = 8
N, B, U, DIN = 10000, 16, 64, 2
D = DIN + U                      # 66
K = D + 1                        # 67 (ones row for biases)
KP = 72                          # K padded to a multiple of 8 (DMA speed)
NPC = N // NCORES                # 1250 nodes/core
NPC_PAD = 1280                   # 10 blocks of 128
NBLK = NPC_PAD // 128            # 10 dest blocks
ROWS = B * NPC_PAD               # 20480
CHUNK_ROWS = 16 * 128            # one block of nodes x all batches
W = B * U                        # 1024 gather row width
AGS = 1                          # allgather split count (launch overhead
                                 # ~12us/collective dwarfs overlap gains)
AG_BOUNDS = [round(i * NBLK / AGS) for i in range(AGS + 1)]
GROUPS = ((0, 3), (3, 6), (6, 9), (9, 10))   # dest-block ranges per gather


def _ytab_row(c, nl):
    """y_tab row index of node (core c, local slot nl) after the split
    AllGather: piece p's output is [rank0 blocks j0:j1, rank1 blocks
    j0:j1, ...] concatenated."""
    j = nl // 128
    si = nl % 128
    bounds = np.asarray(AG_BOUNDS)
    p = np.searchsorted(bounds, j, side="right") - 1
    j0 = bounds[p]
    j1 = bounds[p + 1]
    return (j0 * NCORES + c * (j1 - j0) + (j - j0)) * 128 + si


def build_kernel(nch: tuple[int, ...], stage: int = 5,
                 r_a: int = 1, r_ag: int = 1, r_bc: int = 1,
                 nq: int = 2, no_gather: bool = False):
    """nch[g] = number of 256-slot source chunks of gather group g (same
    for all cores; per-core shortfall is padded with idx 0 / zero oh).
    stage: 1=phase A only, 2=+allgather, 3=+gathers, 4=full
    r_a/r_ag/r_bc: repetition counts (hardware For_i loops / replicated
    collective) for wall-clock-difference timing; correctness needs 1."""
    assert len(nch) == len(GROUPS)
    tot_idx16 = sum(c * 16 for c in nch)
    tot_pairs = sum(c * (j1 - j0) for c, (j0, j1) in zip(nch, GROUPS))

    nc = bacc.Bacc("TRN2", target_bir_lowering=False, debug=False,
                   num_devices=NCORES, num_swdge_queues=nq)

    # ---- I/O ----
    catT_in = nc.dram_tensor("catT", [KP, ROWS], BF16, kind="ExternalInput")
    hxb_in = nc.dram_tensor("hx_blk", [128, NBLK * W], BF16,
                            kind="ExternalInput")
    wfc_in = nc.dram_tensor("wfc", [KP, 128], BF16, kind="ExternalInput")
    wg_in = nc.dram_tensor("wg", [KP, 128], BF16, kind="ExternalInput")
    idx_in = nc.dram_tensor("idxw", [128, tot_idx16], I16,
                            kind="ExternalInput")
    oh_in = nc.dram_tensor("oh", [128, tot_pairs * 256], FP8,
                           kind="ExternalInput")
    out_dram = nc.dram_tensor("out", [NBLK, 128, B, U], BF16,
                              kind="ExternalOutput")

    # ---- internal DRAM (collective) ----
    y_loc = nc.dram_tensor("y_loc", [NBLK * 128, W], FP8, kind="Internal")
    y_tab = nc.dram_tensor("y_tab", [NBLK * NCORES * 128, W], FP8,
                           kind="Internal", addr_space="Shared")

    with tile.TileContext(nc) as tc:
        with (
            tc.tile_pool(name="persist", bufs=1) as pp,
            tc.tile_pool(name="pa", bufs=1) as pa,
        ):
            wfc = pp.tile([KP, 128], BF16)
            nc.sync.dma_start(wfc[:], wfc_in[:])
            wg = pp.tile([KP, 128], BF16)
            nc.sync.dma_start(wg[:], wg_in[:])
            usb = pp.tile([128, NBLK * W], BF16)    # u LOGITS, 20KB/part
            catT = pa.tile([KP, ROWS], BF16)
            # prefetched for phase B/C on the idle Pool engine (SP/ACT
            # rings carry phase A's catT/y traffic)
            oh_sb = pp.tile([128, tot_pairs * 256], FP8)
            nc.gpsimd.dma_start(oh_sb[:], oh_in[:])
            idx_sb = pp.tile([128, tot_idx16], I16)
            nc.gpsimd.dma_start(idx_sb[:], idx_in[:])
            hx_sb = pp.tile([128, NBLK * W], BF16)
            nc.gpsimd.dma_start(hx_sb[:], hxb_in[:])

            # ================= PHASE A =================
            y_writes = []
            with (
                tc.tile_pool(name="pys", bufs=1) as pys,
                tc.tile_pool(name="pa_sig", bufs=4) as psig,
                tc.tile_pool(name="ps_r", bufs=2, space="PSUM") as ps_r,
                tc.tile_pool(name="ps_y", bufs=4, space="PSUM") as ps_y,
            ):
                ystage = pys.tile([128, NBLK * W], FP8)
                with (tc.For_i(0, r_a, 1) if r_a > 1 else nullcontext()):
                    # catT load split so block 0 compute starts early
                    for c0, c1 in ((0, 4096), (4096, 12288), (12288, ROWS)):
                        nc.sync.dma_start(catT[:, c0:c1], catT_in[:, c0:c1])
                    for blk in range(NBLK):
                        boff = blk * CHUNK_ROWS
                        for half in range(2):
                            hsl = slice(boff + half * 1024,
                                        boff + (half + 1) * 1024)
                            # r AND u logits in one pass: out rows 0:64
                            # are r, 64:128 are u (wfc cols 0:128)
                            pr = ps_r.tile([128, 1024], F32)
                            for g in range(2):
                                sl = slice(boff + (half * 2 + g) * 512,
                                           boff + (half * 2 + g + 1) * 512)
                                nc.tensor.matmul(
                                    pr[:, g * 512:(g + 1) * 512],
                                    wfc[:, 0:128], catT[:, sl],
                                    start=True, stop=True)
                            sig = psig.tile([128, 1024], BF16, tag="sig")
                            nc.scalar.activation(
                                sig[:], pr[:],
                                mybir.ActivationFunctionType.Sigmoid)
                            # r*h in place
                            nc.vector.tensor_mul(
                                catT[0:U, hsl], sig[0:U, :], catT[0:U, hsl])
                            # sigmoided u rows -> usb[node, (b, u)] via an
                            # xbar DMA transpose (ACT ring, idle in A):
                            # logical row b*128+node lands at partition
                            # node, mid-index b
                            nc.scalar.dma_start_transpose(
                                usb[:, blk * W + half * 512:
                                     blk * W + (half + 1) * 512]
                                .rearrange("p (t f) -> p t f", f=U),
                                sig[U:128, :])
                            # y: 8 tiles x [node, y] share one PSUM bank
                            pyy = ps_y.tile([128, 512], F32)
                            for i in range(8):
                                b = half * 8 + i
                                tsl = slice(boff + b * 128,
                                            boff + (b + 1) * 128)
                                nc.tensor.matmul(
                                    pyy[:, i * U:(i + 1) * U],
                                    catT[:, tsl], wg[:, 0:U],
                                    start=True, stop=True)
                            nc.vector.tensor_copy(
                                ystage[:, blk * W + half * 512:
                                       blk * W + (half + 1) * 512], pyy[:])
                        # y_loc written in 2-block pieces during A so
                        # the (single) AllGather can launch immediately
                        if (blk + 1) % 2 == 0:
                            j0, j1 = blk - 1, blk + 1
                            ydma = nc.sync.dma_start(
                                y_loc[j0 * 128: j1 * 128, :]
                                .rearrange("(j n) w -> n j w", n=128),
                                ystage[:, j0 * W: j1 * W]
                                .rearrange("n (j w) -> n j w", w=W))
                            y_writes.append(ydma)

                # ============ ALLGATHER (split, overlaps A) ============
                ccs = []
                if stage >= 2:
                    prev_cc = None
                    for rep in range(r_ag):
                        for p in range(AGS):
                            j0, j1 = AG_BOUNDS[p], AG_BOUNDS[p + 1]
                            cc = nc.gpsimd.collective_compute(
                                "AllGather", mybir.AluOpType.bypass,
                                replica_groups=[list(range(NCORES))],
                                ins=[y_loc[j0 * 128: j1 * 128, :]],
                                outs=[y_tab[j0 * NCORES * 128:
                                            j1 * NCORES * 128, :]],
                            )
                            ccs.append(cc)
                            if r_ag > 1 and prev_cc is not None:
                                add_dep_helper(cc.ins, prev_cc.ins,
                                               sync=True,
                                               reason="serialize ag reps")
                            prev_cc = cc
                            if r_a == 1 and r_ag == 1:
                                for yw in y_writes:
                                    add_dep_helper(cc.ins, yw.ins,
                                                   sync=True,
                                                   reason="allgather reads y_loc")

            # ================= PHASE B + C =================
            with (
                tc.tile_pool(name="pg", bufs=3) as pg,
                tc.tile_pool(name="pc", bufs=2) as pcl,
                tc.tile_pool(name="ps_b", bufs=4, space="PSUM") as ps_b,
                tc.For_i(0, r_bc, 1) if r_bc > 1 else nullcontext(),
            ):
                pair_off = 0
                idx_off = 0
                gq = 0
                for grp_i, (j0, j1) in enumerate(GROUPS if stage >= 3 else ()):
                    ngb = j1 - j0
                    nchg = nch[grp_i]
                    # split the group's gather into halves: finer
                    # gather/matmul overlap and half-size gt tiles
                    halves = ([(0, nchg)] if nchg <= 4 else
                              [(0, nchg // 2), (nchg // 2, nchg)])
                    gts = []
                    for (h0, h1) in halves:
                        nidx = (h1 - h0) * 256
                        gt = pg.tile([128, (h1 - h0) * 2, W], FP8, tag="G")
                        if no_gather:   # timing probe: contiguous fill
                            nc.sync.dma_start(   # same bytes, no descs
                                gt[:],
                                y_tab[h0 * 256: h1 * 256]
                                .rearrange("(r p) w -> p r w", p=128))
                            gts.append((h0, h1, gt))
                            continue
                        gather = nc.gpsimd.dma_gather(
                            out_ap=gt[:],
                            in_ap=y_tab[:],
                            idxs_ap=idx_sb[:, idx_off + h0 * 16:
                                           idx_off + h1 * 16],
                            num_idxs=nidx,
                            num_idxs_reg=nidx,
                            elem_size=W,
                            single_packet=False,
                            queue_num=gq % nq,
                        )
                        gq += 1
                        gts.append((h0, h1, gt))
                        if r_bc == 1 and r_ag == 1 and stage >= 2:
                            for cc in ccs:
                                add_dep_helper(
                                    gather.ins, cc.ins, sync=True,
                                    reason="gather reads allgathered y_tab")
                    oh0 = pair_off
                    pair_off += nchg * ngb
                    idx_off += nchg * 16
                    if stage < 4:
                        continue
                    px1s = [ps_b.tile([128, W], F32, name=f"px1_{jj}",
                                      tag="px1")
                            for jj in range(ngb)]
                    for (h0, h1, gt) in gts:
                        for d in range(h0, h1):
                            gsl = slice(2 * (d - h0), 2 * (d - h0) + 2)
                            for jj in range(ngb):
                                ot = oh_sb[
                                    :, (oh0 + d * ngb + jj) * 256:
                                       (oh0 + d * ngb + jj + 1) * 256] \
                                    .rearrange("p (k f) -> p k f", k=2)
                                first = d == 0
                                nc.tensor.matmul(
                                    px1s[jj][:, 0:512], ot,
                                    gt[:, gsl, 0:512],
                                    start=first, stop=False,
                                    perf_mode=mybir.MatmulPerfMode.DoubleRow,
                                    skip_group_check=True)
                                nc.tensor.matmul(
                                    px1s[jj][:, 512:1024], ot,
                                    gt[:, gsl, 512:1024],
                                    start=first, stop=False,
                                    perf_mode=mybir.MatmulPerfMode.DoubleRow,
                                    skip_group_check=True)
                    outg = pcl.tile([128, ngb, W], BF16, tag="outg")
                    for jj in range(ngb):
                        j = j0 + jj
                        # Z0 joins the same accumulation straight from the
                        # persisted (post r*h) catT columns
                        for b in range(B):
                            tsl = slice(j * CHUNK_ROWS + b * 128,
                                        j * CHUNK_ROWS + (b + 1) * 128)
                            nc.tensor.matmul(
                                px1s[jj][:, b * U:(b + 1) * U],
                                catT[:, tsl], wg[:, U:128],
                                start=False, stop=True,
                                skip_group_check=True)
                        # ---- phase C for block j ----
                        ct = pcl.tile([128, W], BF16, tag="c")
                        nc.scalar.activation(
                            ct[:], px1s[jj][:],
                            mybir.ActivationFunctionType.Tanh)
                        dt_ = pcl.tile([128, W], BF16, tag="d")
                        nc.vector.tensor_sub(
                            dt_[:], hx_sb[:, j * W:(j + 1) * W], ct[:])
                        # usb already holds sigmoided u
                        nc.vector.tensor_mul(
                            dt_[:], dt_[:], usb[:, j * W:(j + 1) * W])
                        nc.vector.tensor_add(outg[:, jj, :], dt_[:], ct[:])
                    nc.sync.dma_start(
                        out_dram[j0:j1]
                        .rearrange("j n b u -> n j (b u)"),
                        outg[:])

    nc.compile()
    return nc


# ---------------- host side ----------------

def prep_inputs(inputs, hx, rows, cols, vals, W_fc, b_fc, W_g, b_g):
    """Build the 8 per-core input maps + the dedup gather geometry."""
    xi = np.asarray(inputs).reshape(B, N, DIN)
    h = np.asarray(hx).reshape(B, N, U)
    rows = np.asarray(rows); cols = np.asarray(cols); vals = np.asarray(vals)

    core_of = rows // NPC
    # ---- per-core node->block balancing (~2048 edges/block) ----
    perms = []          # perms[k][slot] = original local node (or -1 pad)
    slot_of = np.full((NCORES, NPC), -1, np.int64)  # local node -> slot
    for k in range(NCORES):
        deg = np.bincount(rows[core_of == k] - k * NPC, minlength=NPC)
        order = np.argsort(-deg, kind="stable")
        blk_edges = np.zeros(NBLK, np.int64)
        blk_nodes = [[] for _ in range(NBLK)]
        for n in order:
            best, be = -1, 1 << 60
            for j in range(NBLK):
                if len(blk_nodes[j]) < 128 and blk_edges[j] < be:
                    best, be = j, blk_edges[j]
            blk_nodes[best].append(n)
            blk_edges[best] += deg[n]
        perm = np.full(NPC_PAD, -1, np.int64)
        for j in range(NBLK):
            nodes = blk_nodes[j]
            perm[j * 128: j * 128 + len(nodes)] = nodes
            for si, n in enumerate(nodes):
                slot_of[k, n] = j * 128 + si
        perms.append(perm)

    # source (global node) -> y_tab row, via the owning core's slot
    src_core = np.arange(N) // NPC
    src_slot = slot_of[src_core, np.arange(N) % NPC]
    ytab_of_node = _ytab_row(src_core, src_slot).astype(np.int16)

    # ---- per-core per-group edge lists, dedup geometry ----
    glists = []       # glists[k][g] = (uniq_cols, slot_idx, dest_local, val)
    for k in range(NCORES):
        m = core_of == k
        slot = slot_of[k, rows[m] - k * NPC]
        c_l = cols[m]; v_l = vals[m]
        blk = slot // 128
        per_g = []
        for g, (j0, j1) in enumerate(GROUPS):
            gm = (blk >= j0) & (blk < j1)
            cg = c_l[gm]
            uniq, inv = np.unique(cg, return_inverse=True)
            dest = (blk[gm] - j0) * 128 + (slot[gm] % 128)
            per_g.append((uniq, inv, dest, v_l[gm]))
        glists.append(per_g)

    nch = tuple(
        max(1, int(-(-max(len(glists[k][g][0]) for k in range(NCORES))
                    // 256)))
        for g in range(len(GROUPS)))
    tot_idx16 = sum(c * 16 for c in nch)
    tot_pairs = sum(c * (j1 - j0) for c, (j0, j1) in zip(nch, GROUPS))

    # feature order everywhere: [h (0:64), xi (64:66), ones (66)]
    perm_f = np.concatenate([np.arange(DIN, D), np.arange(DIN)])
    wfc_ext = np.zeros((KP, 128), np.float32)
    wfc_ext[:D] = np.asarray(W_fc)[perm_f]
    wfc_ext[D] = np.asarray(b_fc)
    wg = np.asarray(W_g).reshape(D, 2, U)
    wg_comb = np.zeros((KP, 128), np.float32)
    wg_comb[:D, :U] = wg[perm_f, 1, :]       # odd rows -> Y
    wg_comb[:D, U:] = wg[perm_f, 0, :]       # even rows -> Z0
    wg_comb[D, U:] = np.asarray(b_g)         # b_g into Z0

    in_maps = []
    for k in range(NCORES):
        sl = slice(k * NPC, (k + 1) * NPC)
        perm = perms[k]
        valid = perm >= 0
        xi_p = np.zeros((B, NPC_PAD, DIN), np.float32)
        xi_p[:, valid] = xi[:, sl][:, perm[valid]]
        h_p = np.zeros((B, NPC_PAD, U), np.float32)
        h_p[:, valid] = h[:, sl][:, perm[valid]]
        # rows ordered (blk, b, nl): tile t = blk*16 + b
        catT = np.zeros((KP, ROWS), np.float32)
        catT[0:U] = (h_p.reshape(B, NBLK, 128, U)
                     .transpose(3, 1, 0, 2).reshape(U, ROWS))
        catT[U:D] = (xi_p.reshape(B, NBLK, 128, DIN)
                     .transpose(3, 1, 0, 2).reshape(DIN, ROWS))
        catT[D] = 1.0
        hx_blk = (h_p.reshape(B, NBLK, 128, U)
                  .transpose(2, 1, 0, 3).reshape(128, NBLK * B * U))

        idx_w = np.zeros((128, tot_idx16), np.int16)
        oh32 = np.zeros((128, tot_pairs * 256), np.float32)
        ioff = 0
        pair_off = 0
        for g, (j0, j1) in enumerate(GROUPS):
            ngb = j1 - j0
            uniq, inv, dest, v_g = glists[k][g]
            d_ = inv // 256
            sc = inv % 256
            colpos = ((pair_off + d_ * ngb + dest // 128) * 256
                      + (sc // 128) * 128 + dest % 128)
            np.add.at(oh32, (sc % 128, colpos), v_g)
            idx = np.zeros(nch[g] * 256, np.int16)
            idx[:len(uniq)] = ytab_of_node[uniq]
            wrap = idx.reshape(nch[g] * 16, 16).T        # (16, nidx/16)
            idx_w[:, ioff: ioff + nch[g] * 16] = np.tile(wrap, (8, 1))
            ioff += nch[g] * 16
            pair_off += nch[g] * ngb

        in_maps.append({
            "catT": catT.astype(ml_dtypes.bfloat16),
            "hx_blk": hx_blk.astype(ml_dtypes.bfloat16),
            "wfc": wfc_ext.astype(ml_dtypes.bfloat16),
            "wg": wg_comb.astype(ml_dtypes.bfloat16),
            "idxw": idx_w,
            "oh": oh32.astype(ml_dtypes.float8_e4m3),
        })
    return nch, in_maps, perms


_CACHE: dict = {}


def assemble_out(results, perms):
    """results[k]['out'] is (NBLK, 128, B, U) bf16; -> (B, N*U) f32."""
    full = np.empty((N, B, U), np.float32)
    for k in range(NCORES):
        o = results[k]["out"].reshape(NPC_PAD, B, U).astype(np.float32)
        perm = perms[k]
        valid = perm >= 0
        full[k * NPC + perm[valid]] = o[valid]
    return full.transpose(1, 0, 2).reshape(B, N * U)


def run(inputs, hx, rows, cols, vals, W_fc, b_fc, W_g, b_g):
    nch, in_maps, perms = prep_inputs(inputs, hx, rows, cols, vals,
                                      W_fc, b_fc, W_g, b_g)
    if nch not in _CACHE:
        _CACHE[nch] = build_kernel(nch)
    nc = _CACHE[nch]
    res = run_bass_kernel_spmd(nc, in_maps, core_ids=list(range(NCORES)))
    return assemble_out(res.results, perms)


def kernel(inputs, hx, rows, cols, vals, W_fc, b_fc, W_g, b_g):
    """Harness entry: full (unsharded) inputs -> full output (B, N*U)."""
    out = run(inputs, hx, rows, cols, vals, W_fc, b_fc, W_g, b_g)
    return out.astype(np.float32)


# revision 30
# speedup vs baseline: 1.0940x; 1.0940x over previous
"""GCGRUCell Trainium2 kernel — 8-core SPMD, fp8 aggregation path.

Math (per reference):
  value = sigmoid(cat([x, h]) @ W_fc + b_fc);  r, u = split(value)
  X0 = cat([x, r*h])                                (B, N, D)   D=66
  Y  = X0 @ Wg_odd                                  (B, N, U)
  X1 = S @ Y          (segment_sum over E edges, commuted with Wg_odd)
  c  = tanh(X0 @ Wg_even + X1 + b_g)
  out = u * hx + (1-u) * c
Sharding: nodes 1250/core (10 dest blocks of 128, greedily balanced);
all B=16 batches per core for the aggregation (gathered rows are
B*U=1024B fp8).

The gather is per-DESCRIPTOR-latency bound (~4.8 ns/desc aggregate +
~3.3 ns/KB measured; sorted indices / more SWDGE queues don't help), so
phase B dedups sources per gather GROUP of 3 dest blocks: each group's
~6k edges reference only ~4.6k distinct sources; one descriptor per
DISTINCT source, and the per-(chunk, dest-block) one-hot fp8 DoubleRow
matrices absorb the duplicate fan-out (a slot may carry several dests /
accumulated parallel edges).  Each group's gather is split in halves on
alternating queues, and gt runs bufs=3: with bufs=2 the next-next
gather's WAR on the tile serializes the pipeline (+40us measured).  3
PSUM accumulators (one per block in the group) live through the group's
chunks; Z0 joins the same accumulation from the persisted catT (16 bf16
matmuls per block, 64-wide moving so the ld dominates but PE still
hides under the gather).  oh/idx/hx are prefetched into SBUF during
phase A on the Pool queue — streaming them in phase B instead puts
their ~8MB in the gather's DMA-engine window (global DMA-engine
contention, not per-ring, is what matters).

Phase A per block/half: ONE [128, 1024] matmul pass makes r AND u
logits (wfc cols 0:128), one sigmoid; r*h in place into catT; the
sigmoided u rows leave via an xbar DMA-transpose (ACT ring) straight
into usb[node, (b, u)] — no transposing u-matmuls, no phase-C sigmoid.
y matmuls transpose per batch tile with a DVE PSUM->fp8 copy.  Y rows
go to y_loc in 2-block pieces during A; ONE AllGather moves y_loc into
the (10240, 1024) fp8 y_tab: collective launch overhead (~12us each)
dwarfs any split-for-overlap gain (5-way split AG measured 75us vs
27us for one).

Row order: tile t in [0,16) = batch, 128 nodes per tile; feature order
[h (0:64), xi (64:66), ones (66)]; biases ride the ones row.  Measured
(loop-differenced phases, noisy +-20%): A ~50-60us, AG ~27us, BC
~125-130us (gathers alone ~120); sum 177-250us vs 309us for the
session-start baseline under the identical protocol.
"""

from contextlib import nullcontext

import numpy as np
import ml_dtypes
import concourse.bass as bass
import concourse.bacc as bacc
import concourse.mybir as mybir
import concourse.tile as tile
from bass_rust import add_dep_helper
from concourse.bass_utils import run_bass_kernel_spmd

F32 = mybir.dt.float32
BF16 = mybir.dt.bfloat16
FP8 = mybir.dt.float8e4
I16 = mybir.dt.int16

NCORES = 8
N, B, U, DIN = 10000, 16, 64, 2
D = DIN + U                      # 66
K = D + 1                        # 67 (ones row for biases)
KP = 72                          # K padded to a multiple of 8 (DMA speed)
NPC = N // NCORES                # 1250 nodes/core
NPC_PAD = 1280                   # 10 blocks of 128
NBLK = NPC_PAD // 128            # 10 dest blocks
ROWS = B * NPC_PAD               # 20480
CHUNK_ROWS = 16 * 128            # one block of nodes x all batches
W = B * U                        # 1024 gather row width
AGS = 1                          # allgather split count (launch overhead
                                 # ~12us/collective dwarfs overlap gains)
AG_BOUNDS = [round(i * NBLK / AGS) for i in range(AGS + 1)]
GROUPS = ((0, 3), (3, 6), (6, 9), (9, 10))   # dest-block ranges per gather


def _ytab_row(c, nl):
    """y_tab row index of node (core c, local slot nl) after the split
    AllGather: piece p's output is [rank0 blocks j0:j1, rank1 blocks
    j0:j1, ...] concatenated."""
    j = nl // 128
    si = nl % 128
    bounds = np.asarray(AG_BOUNDS)
    p = np.searchsorted(bounds, j, side="right") - 1
    j0 = bounds[p]
    j1 = bounds[p + 1]
    return (j0 * NCORES + c * (j1 - j0) + (j - j0)) * 128 + si


def build_kernel(nch: tuple[int, ...], stage: int = 5,
                 r_a: int = 1, r_ag: int = 1, r_bc: int = 1,
                 nq: int = 2, no_gather: bool = False):
    """nch[g] = number of 256-slot source chunks of gather group g (same
    for all cores; per-core shortfall is padded with idx 0 / zero oh).
    stage: 1=phase A only, 2=+allgather, 3=+gathers, 4=full
    r_a/r_ag/r_bc: repetition counts (hardware For_i loops / replicated
    collective) for wall-clock-difference timing; correctness needs 1."""
    assert len(nch) == len(GROUPS)
    tot_idx16 = sum(c * 16 for c in nch)
    tot_pairs = sum(c * (j1 - j0) for c, (j0, j1) in zip(nch, GROUPS))

    nc = bacc.Bacc("TRN2", target_bir_lowering=False, debug=False,
                   num_devices=NCORES, num_swdge_queues=nq)

    # ---- I/O ----
    catT_in = nc.dram_tensor("catT", [KP, ROWS], BF16, kind="ExternalInput")
    hxb_in = nc.dram_tensor("hx_blk", [128, NBLK * W], BF16,
                            kind="ExternalInput")
    wfc_in = nc.dram_tensor("wfc", [KP, 128], BF16, kind="ExternalInput")
    wg_in = nc.dram_tensor("wg", [KP, 128], BF16, kind="ExternalInput")
    idx_in = nc.dram_tensor("idxw", [128, tot_idx16], I16,
                            kind="ExternalInput")
    oh_in = nc.dram_tensor("oh", [128, tot_pairs * 256], FP8,
                           kind="ExternalInput")
    out_dram = nc.dram_tensor("out", [NBLK, 128, B, U], BF16,
                              kind="ExternalOutput")

    # ---- internal DRAM (collective) ----
    y_loc = nc.dram_tensor("y_loc", [NBLK * 128, W], FP8, kind="Internal")
    y_tab = nc.dram_tensor("y_tab", [NBLK * NCORES * 128, W], FP8,
                           kind="Internal", addr_space="Shared")

    with tile.TileContext(nc) as tc:
        with (
            tc.tile_pool(name="persist", bufs=1) as pp,
            tc.tile_pool(name="pa", bufs=1) as pa,
        ):
            wfc = pp.tile([KP, 128], BF16)
            nc.sync.dma_start(wfc[:], wfc_in[:])
            wg = pp.tile([KP, 128], BF16)
            nc.sync.dma_start(wg[:], wg_in[:])
            usb = pp.tile([128, NBLK * W], BF16)    # u LOGITS, 20KB/part
            catT = pa.tile([KP, ROWS], BF16)
            # prefetched for phase B/C on the idle Pool engine (SP/ACT
            # rings carry phase A's catT/y traffic)
            oh_sb = pp.tile([128, tot_pairs * 256], FP8)
            nc.gpsimd.dma_start(oh_sb[:], oh_in[:])
            idx_sb = pp.tile([128, tot_idx16], I16)
            nc.gpsimd.dma_start(idx_sb[:], idx_in[:])
            hx_sb = pp.tile([128, NBLK * W], BF16)
            nc.gpsimd.dma_start(hx_sb[:], hxb_in[:])

            # ================= PHASE A =================
            y_writes = []
            with (
                tc.tile_pool(name="pys", bufs=1) as pys,
                tc.tile_pool(name="pa_sig", bufs=4) as psig,
                tc.tile_pool(name="ps_r", bufs=2, space="PSUM") as ps_r,
                tc.tile_pool(name="ps_y", bufs=4, space="PSUM") as ps_y,
            ):
                ystage = pys.tile([128, NBLK * W], FP8)
                with (tc.For_i(0, r_a, 1) if r_a > 1 else nullcontext()):
                    # catT load split so block 0 compute starts early
                    for c0, c1 in ((0, 4096), (4096, 12288), (12288, ROWS)):
                        nc.sync.dma_start(catT[:, c0:c1], catT_in[:, c0:c1])
                    for blk in range(NBLK):
                        boff = blk * CHUNK_ROWS
                        for half in range(2):
                            hsl = slice(boff + half * 1024,
                                        boff + (half + 1) * 1024)
                            # r AND u logits in one pass: out rows 0:64
                            # are r, 64:128 are u (wfc cols 0:128)
                            pr = ps_r.tile([128, 1024], F32)
                            for g in range(2):
                                sl = slice(boff + (half * 2 + g) * 512,
                                           boff + (half * 2 + g + 1) * 512)
                                nc.tensor.matmul(
                                    pr[:, g * 512:(g + 1) * 512],
                                    wfc[:, 0:128], catT[:, sl],
                                    start=True, stop=True)
                            sig = psig.tile([128, 1024], BF16, tag="sig")
                            nc.scalar.activation(
                                sig[:], pr[:],
                                mybir.ActivationFunctionType.Sigmoid)
                            # r*h in place
                            nc.vector.tensor_mul(
                                catT[0:U, hsl], sig[0:U, :], catT[0:U, hsl])
                            # sigmoided u rows -> usb[node, (b, u)] via an
                            # xbar DMA transpose (ACT ring, idle in A):
                            # logical row b*128+node lands at partition
                            # node, mid-index b
                            nc.scalar.dma_start_transpose(
                                usb[:, blk * W + half * 512:
                                     blk * W + (half + 1) * 512]
                                .rearrange("p (t f) -> p t f", f=U),
                                sig[U:128, :])
                            # y: 8 tiles x [node, y] share one PSUM bank
                            pyy = ps_y.tile([128, 512], F32)
                            for i in range(8):
                                b = half * 8 + i
                                tsl = slice(boff + b * 128,
                                            boff + (b + 1) * 128)
                                nc.tensor.matmul(
                                    pyy[:, i * U:(i + 1) * U],
                                    catT[:, tsl], wg[:, 0:U],
                                    start=True, stop=True)
                            nc.vector.tensor_copy(
                                ystage[:, blk * W + half * 512:
                                       blk * W + (half + 1) * 512], pyy[:])
                        # y_loc written in 2-block pieces during A so
                        # the (single) AllGather can launch immediately
                        if (blk + 1) % 2 == 0:
                            j0, j1 = blk - 1, blk + 1
                            ydma = nc.sync.dma_start(
                                y_loc[j0 * 128: j1 * 128, :]
                                .rearrange("(j n) w -> n j w", n=128),
                                ystage[:, j0 * W: j1 * W]
                                .rearrange("n (j w) -> n j w", w=W))
                            y_writes.append(ydma)

                # ============ ALLGATHER (split, overlaps A) ============
                ccs = []
                if stage >= 2:
                    prev_cc = None
                    for rep in range(r_ag):
                        for p in range(AGS):
                            j0, j1 = AG_BOUNDS[p], AG_BOUNDS[p + 1]
                            cc = nc.gpsimd.collective_compute(
                                "AllGather", mybir.AluOpType.bypass,
                                replica_groups=[list(range(NCORES))],
                                ins=[y_loc[j0 * 128: j1 * 128, :]],
                                outs=[y_tab[j0 * NCORES * 128:
                                            j1 * NCORES * 128, :]],
                            )
                            ccs.append(cc)
                            if r_ag > 1 and prev_cc is not None:
                                add_dep_helper(cc.ins, prev_cc.ins,
                                               sync=True,
                                               reason="serialize ag reps")
                            prev_cc = cc
                            if r_a == 1 and r_ag == 1:
                                for yw in y_writes:
                                    add_dep_helper(cc.ins, yw.ins,
                                                   sync=True,
                                                   reason="allgather reads y_loc")

            # ================= PHASE B + C =================
            with (
                tc.tile_pool(name="pg", bufs=3) as pg,
                tc.tile_pool(name="pc", bufs=2) as pcl,
                tc.tile_pool(name="ps_b", bufs=4, space="PSUM") as ps_b,
                tc.For_i(0, r_bc, 1) if r_bc > 1 else nullcontext(),
            ):
                pair_off = 0
                idx_off = 0
                gq = 0
                for grp_i, (j0, j1) in enumerate(GROUPS if stage >= 3 else ()):
                    ngb = j1 - j0
                    nchg = nch[grp_i]
                    # split the group's gather into halves: finer
                    # gather/matmul overlap and half-size gt tiles
                    halves = ([(0, nchg)] if nchg <= 4 else
                              [(0, nchg // 2), (nchg // 2, nchg)])
                    gts = []
                    for (h0, h1) in halves:
                        nidx = (h1 - h0) * 256
                        gt = pg.tile([128, (h1 - h0) * 2, W], FP8, tag="G")
                        if no_gather:   # timing probe: contiguous fill
                            nc.sync.dma_start(   # same bytes, no descs
                                gt[:],
                                y_tab[h0 * 256: h1 * 256]
                                .rearrange("(r p) w -> p r w", p=128))
                            gts.append((h0, h1, gt))
                            continue
                        gather = nc.gpsimd.dma_gather(
                            out_ap=gt[:],
                            in_ap=y_tab[:],
                            idxs_ap=idx_sb[:, idx_off + h0 * 16:
                                           idx_off + h1 * 16],
                            num_idxs=nidx,
                            num_idxs_reg=nidx,
                            elem_size=W,
                            single_packet=False,
                            queue_num=gq % nq,
                        )
                        gq += 1
                        gts.append((h0, h1, gt))
                        if r_bc == 1 and r_ag == 1 and stage >= 2:
                            for cc in ccs:
                                add_dep_helper(
                                    gather.ins, cc.ins, sync=True,
                                    reason="gather reads allgathered y_tab")
                    oh0 = pair_off
                    pair_off += nchg * ngb
                    idx_off += nchg * 16
                    if stage < 4:
                        continue
                    px1s = [ps_b.tile([128, W], F32, name=f"px1_{jj}",
                                      tag="px1")
                            for jj in range(ngb)]
                    for (h0, h1, gt) in gts:
                        for d in range(h0, h1):
                            gsl = slice(2 * (d - h0), 2 * (d - h0) + 2)
                            for jj in range(ngb):
                                ot = oh_sb[
                                    :, (oh0 + d * ngb + jj) * 256:
                                       (oh0 + d * ngb + jj + 1) * 256] \
                                    .rearrange("p (k f) -> p k f", k=2)
                                first = d == 0
                                nc.tensor.matmul(
                                    px1s[jj][:, 0:512], ot,
                                    gt[:, gsl, 0:512],
                                    start=first, stop=False,
                                    perf_mode=mybir.MatmulPerfMode.DoubleRow,
                                    skip_group_check=True)
                                nc.tensor.matmul(
                                    px1s[jj][:, 512:1024], ot,
                                    gt[:, gsl, 512:1024],
                                    start=first, stop=False,
                                    perf_mode=mybir.MatmulPerfMode.DoubleRow,
                                    skip_group_check=True)
                    outg = pcl.tile([128, ngb, W], BF16, tag="outg")
                    for jj in range(ngb):
                        j = j0 + jj
                        # Z0 joins the same accumulation straight from the
                        # persisted (post r*h) catT columns
                        for b in range(B):
                            tsl = slice(j * CHUNK_ROWS + b * 128,
                                        j * CHUNK_ROWS + (b + 1) * 128)
                            nc.tensor.matmul(
                                px1s[jj][:, b * U:(b + 1) * U],
                                catT[:, tsl], wg[:, U:128],
                                start=False, stop=True,
                                skip_group_check=True)
                        # ---- phase C for block j ----
                        ct = pcl.tile([128, W], BF16, tag="c")
                        nc.scalar.activation(
                            ct[:], px1s[jj][:],
                            mybir.ActivationFunctionType.Tanh)
                        dt_ = pcl.tile([128, W], BF16, tag="d")
                        nc.vector.tensor_sub(
                            dt_[:], hx_sb[:, j * W:(j + 1) * W], ct[:])
                        # usb already holds sigmoided u
                        nc.vector.tensor_mul(
                            dt_[:], dt_[:], usb[:, j * W:(j + 1) * W])
                        nc.vector.tensor_add(outg[:, jj, :], dt_[:], ct[:])
                    nc.sync.dma_start(
                        out_dram[j0:j1]
                        .rearrange("j n b u -> n j (b u)"),
                        outg[:])

    nc.compile()
    return nc


# ---------------- host side ----------------

def prep_inputs(inputs, hx, rows, cols, vals, W_fc, b_fc, W_g, b_g):
    """Build the 8 per-core input maps + the dedup gather geometry."""
    xi = np.asarray(inputs).reshape(B, N, DIN)
    h = np.asarray(hx).reshape(B, N, U)
    rows = np.asarray(rows); cols = np.asarray(cols); vals = np.asarray(vals)

    core_of = rows // NPC
    # ---- per-core node->block balancing (~2048 edges/block) ----
    perms = []          # perms[k][slot] = original local node (or -1 pad)
    slot_of = np.full((NCORES, NPC), -1, np.int64)  # local node -> slot
    for k in range(NCORES):
        deg = np.bincount(rows[core_of == k] - k * NPC, minlength=NPC)
        order = np.argsort(-deg, kind="stable")
        blk_edges = np.zeros(NBLK, np.int64)
        blk_nodes = [[] for _ in range(NBLK)]
        for n in order:
            best, be = -1, 1 << 60
            for j in range(NBLK):
                if len(blk_nodes[j]) < 128 and blk_edges[j] < be:
                    best, be = j, blk_edges[j]
            blk_nodes[best].append(n)
            blk_edges[best] += deg[n]
        perm = np.full(NPC_PAD, -1, np.int64)
        for j in range(NBLK):
            nodes = blk_nodes[j]
            perm[j * 128: j * 128 + len(nodes)] = nodes
            for si, n in enumerate(nodes):
                slot_of[k, n] = j * 128 + si
        perms.append(perm)

    # source (global node) -> y_tab row, via the owning core's slot
    src_core = np.arange(N) // NPC
    src_slot = slot_of[src_core, np.arange(N) % NPC]
    ytab_of_node = _ytab_row(src_core, src_slot).astype(np.int16)

    # ---- per-core per-group edge lists, dedup geometry ----
    glists = []       # glists[k][g] = (uniq_cols, slot_idx, dest_local, val)
    for k in range(NCORES):
        m = core_of == k
        slot = slot_of[k, rows[m] - k * NPC]
        c_l = cols[m]; v_l = vals[m]
        blk = slot // 128
        per_g = []
        for g, (j0, j1) in enumerate(GROUPS):
            gm = (blk >= j0) & (blk < j1)
            cg = c_l[gm]
            uniq, inv = np.unique(cg, return_inverse=True)
            dest = (blk[gm] - j0) * 128 + (slot[gm] % 128)
            per_g.append((uniq, inv, dest, v_l[gm]))
        glists.append(per_g)

    nch = tuple(
        max(1, int(-(-max(len(glists[k][g][0]) for k in range(NCORES))
                    // 256)))
        for g in range(len(GROUPS)))
    tot_idx16 = sum(c * 16 for c in nch)
    tot_pairs = sum(c * (j1 - j0) for c, (j0, j1) in zip(nch, GROUPS))

    # feature order everywhere: [h (0:64), xi (64:66), ones (66)]
    perm_f = np.concatenate([np.arange(DIN, D), np.arange(DIN)])
    wfc_ext = np.zeros((KP, 128), np.float32)
    wfc_ext[:D] = np.asarray(W_fc)[perm_f]
    wfc_ext[D] = np.asarray(b_fc)
    wg = np.asarray(W_g).reshape(D, 2, U)
    wg_comb = np.zeros((KP, 128), np.float32)
    wg_comb[:D, :U] = wg[perm_f, 1, :]       # odd rows -> Y
    wg_comb[:D, U:] = wg[perm_f, 0, :]       # even rows -> Z0
    wg_comb[D, U:] = np.asarray(b_g)         # b_g into Z0

    in_maps = []
    for k in range(NCORES):
        sl = slice(k * NPC, (k + 1) * NPC)
        perm = perms[k]
        valid = perm >= 0
        xi_p = np.zeros((B, NPC_PAD, DIN), np.float32)
        xi_p[:, valid] = xi[:, sl][:, perm[valid]]
        h_p = np.zeros((B, NPC_PAD, U), np.float32)
        h_p[:, valid] = h[:, sl][:, perm[valid]]
        # rows ordered (blk, b, nl): tile t = blk*16 + b
        catT = np.zeros((KP, ROWS), np.float32)
        catT[0:U] = (h_p.reshape(B, NBLK, 128, U)
                     .transpose(3, 1, 0, 2).reshape(U, ROWS))
        catT[U:D] = (xi_p.reshape(B, NBLK, 128, DIN)
                     .transpose(3, 1, 0, 2).reshape(DIN, ROWS))
        catT[D] = 1.0
        hx_blk = (h_p.reshape(B, NBLK, 128, U)
                  .transpose(2, 1, 0, 3).reshape(128, NBLK * B * U))

        idx_w = np.zeros((128, tot_idx16), np.int16)
        oh32 = np.zeros((128, tot_pairs * 256), np.float32)
        ioff = 0
        pair_off = 0
        for g, (j0, j1) in enumerate(GROUPS):
            ngb = j1 - j0
            uniq, inv, dest, v_g = glists[k][g]
            d_ = inv // 256
            sc = inv % 256
            colpos = ((pair_off + d_ * ngb + dest // 128) * 256
                      + (sc // 128) * 128 + dest % 128)
            np.add.at(oh32, (sc % 128, colpos), v_g)
            idx = np.zeros(nch[g] * 256, np.int16)
            idx[:len(uniq)] = ytab_of_node[uniq]
            wrap = idx.reshape(nch[g] * 16, 16).T        # (16, nidx/16)
            idx_w[:, ioff: ioff + nch[g] * 16] = np.tile(wrap, (8, 1))
            ioff += nch[g] * 16
            pair_off += nch[g] * ngb

        in_maps.append({
            "catT": catT.astype(ml_dtypes.bfloat16),
            "hx_blk": hx_blk.astype(ml_dtypes.bfloat16),
            "wfc": wfc_ext.astype(ml_dtypes.bfloat16),
            "wg": wg_comb.astype(ml_dtypes.bfloat16),
            "idxw": idx_w,
            "oh": oh32.astype(ml_dtypes.float8_e4m3),
        })
    return nch, in_maps, perms


_CACHE: dict = {}


def assemble_out(results, perms):
    """results[k]['out'] is (NBLK, 128, B, U) bf16; -> (B, N*U) f32."""
    full = np.empty((N, B, U), np.float32)
    for k in range(NCORES):
        o = results[k]["out"].reshape(NPC_PAD, B, U).astype(np.float32)
        perm = perms[k]
        valid = perm >= 0
        full[k * NPC + perm[valid]] = o[valid]
    return full.transpose(1, 0, 2).reshape(B, N * U)


def run(inputs, hx, rows, cols, vals, W_fc, b_fc, W_g, b_g):
    nch, in_maps, perms = prep_inputs(inputs, hx, rows, cols, vals,
                                      W_fc, b_fc, W_g, b_g)
    if nch not in _CACHE:
        _CACHE[nch] = build_kernel(nch)
    nc = _CACHE[nch]
    res = run_bass_kernel_spmd(nc, in_maps, core_ids=list(range(NCORES)))
    return assemble_out(res.results, perms)


def kernel(inputs, hx, rows, cols, vals, W_fc, b_fc, W_g, b_g):
    """Harness entry: full (unsharded) inputs -> full output (B, N*U)."""
    out = run(inputs, hx, rows, cols, vals, W_fc, b_fc, W_g, b_g)
    return out.astype(np.float32)
